# revision 50
# baseline (speedup 1.0000x reference)
"""Trainium2 Bass kernel for Convpass-swin hypernet fused adapter.

Reference computation (per batch sample):
  h      = relu(x @ Wm1 + bm1)                    # [B,H,W,64]
  prompt = mean_hw(h) @ Wm2 + bm2                 # [B,64]  (mean commutes with matmul)
  wflat  = (emb + prompt) @ Wh + bh               # [B,96*96*9]
  xd     = quickgelu(x @ Wd + bd)                 # [B,H,W,96]
  y      = quickgelu(conv3x3(xd, wflat))          # per-sample dynamic grouped conv
  out    = y @ Wu + bu                            # [B,H,W,384]

Sharding: data-parallel over batch B=64 across 8 cores (8 samples/core),
weights replicated.

Device-side design (host prep is free, only HW exec time counts):
- x arrives pre-transposed [C, pos] in bf16: no PE transposes, half the DMA.
- small weights arrive in two packed tensors (one bf16, one f32) plus wm2,
  so the DGE pipeline isn't serialized by many tiny transfers.
- Wh is augmented with a bh row (pvec carries a matching ones row) and its
  columns are reordered (e, t, o, i) so each DMA chunk is one conv tap; the
  conv runs in two passes (taps 0-3 into an f32 SBUF accumulator while the
  stream continues, taps 4-8 accumulated in PSUM on top) so most of the
  conv overlaps the Wh stream.
- per-tap weight tiles (not one big tensor) keep the tile-granularity
  dependency tracking from serializing conv against later tap writes.
- emission order is queue-aware: engines run in order with a shallow wait
  queue, so work is emitted only once its dependencies are ~2 taps behind
  the Wh stream cursor (xd matmuls fill the early stream, conv pass 0
  slots between taps 4..8).
- PSUM can only be drained by ACT and DVE (GPSIMD may not touch PSUM on
  real HW); drains are balanced across both and the up-projection output
  is staged as bf16 [C, pos] (host de-transposes and upcasts).
- bu rides as a bias row of y_sb against an extra wu row; bm1 is folded
  into the DVE relu-sum via max(h,-bm1)+P*bm1.
"""
import numpy as np
import ml_dtypes

import concourse.bass as bass
import concourse.tile as tile
import concourse.mybir as mybir
from concourse import bacc
from concourse.alu_op_type import AluOpType
from concourse.bass_utils import run_bass_kernel_spmd

F32 = mybir.dt.float32
BF = mybir.dt.bfloat16
FP8 = mybir.dt.float8e4
AF = mybir.ActivationFunctionType
AX = mybir.AxisListType

# problem constants
B, H, W, C = 64, 28, 28, 384
DIM, E, KK = 96, 64, 3
NCORES = 8
BL = B // NCORES          # samples per core
P = H * W                 # 784 positions per sample
HP = H + 2                # padded spatial
NPOS = BL * P             # 6272 positions per core
NTAP = KK * KK            # 9 conv taps
TAPW = DIM * DIM          # 9216 Wh columns per tap (o, i)
WH_COLS = TAPW * NTAP     # 82944

WH_FP8 = False            # fp8 e4m3 Wh costs ~2% rel err: over the gate
WH_SCALE = 256.0 if WH_FP8 else 1.0
WH_DT = FP8 if WH_FP8 else BF
# conv pass structure: pass p covers taps TAP_PASS[p] .. TAP_PASS[p+1]-1
TAP_PASS = (0, 4, 9)
WH_BUFS = 8 if WH_FP8 else 4
# packed small-weight tensors: column offsets
BFP_WM1, BFP_WD, BFP_WU, BFP_N = 0, 192, 480, 864
FP_NBM1, FP_PBM1, FP_BD, FP_BPE, FP_BU, FP_N = 0, 1, 2, 3, 4, 7
EA = E + 1                # contraction rows incl. the bh bias row


def build_nc(stop_after=None, dbg=False):
    nc = bacc.Bacc("TRN2", target_bir_lowering=False, debug=False)

    x_d = nc.dram_tensor("x", [C, NPOS], BF, kind="ExternalInput").ap()
    wpb_d = nc.dram_tensor("wpb", [128, BFP_N], BF, kind="ExternalInput").ap()
    wpf_d = nc.dram_tensor("wpf", [128, FP_N], F32, kind="ExternalInput").ap()
    wh_d = nc.dram_tensor("wh", [EA, WH_COLS], WH_DT, kind="ExternalInput").ap()
    ones_d = nc.dram_tensor("ones", [1, NPOS], BF, kind="ExternalInput").ap()
    wm2_d = nc.dram_tensor("wm2", [E, E], F32, kind="ExternalInput").ap()
    out_d = nc.dram_tensor("out", [C, NPOS], BF, kind="ExternalOutput").ap()
    if dbg:
        pvec_d = nc.dram_tensor("pvec_dbg", [EA, BL], BF, kind="ExternalOutput").ap()
        w_d = nc.dram_tensor("w_dbg", [DIM, NTAP * DIM * BL], BF,
                             kind="ExternalOutput").ap()
        xd_d = nc.dram_tensor("xd_dbg", [DIM, BL * HP * HP], BF,
                              kind="ExternalOutput").ap()
        y_d = nc.dram_tensor("y_dbg", [DIM + 1, BL * P], BF,
                             kind="ExternalOutput").ap()

    with tile.TileContext(nc) as tc:
        with (
            tc.tile_pool(name="const", bufs=1) as cp,
            tc.tile_pool(name="persist", bufs=1) as pp,
            tc.tile_pool(name="hscr", bufs=2) as hscr_p,
            tc.tile_pool(name="wh", bufs=WH_BUFS) as wh_p,
            tc.tile_pool(name="ob", bufs=3) as ob_p,
        ):
            # ---- packed constants + first x sample (split for latency) ----
            xT = pp.tile([128, C // 128, NPOS], BF)       # x in [c, pos]
            xsrc0 = x_d.rearrange("(c3 p) n -> p c3 n", p=128)
            wpb = cp.tile([128, BFP_N], BF)
            nc.sync.dma_start(wpb[:], wpb_d[:])
            nc.sync.dma_start(xT[:, :, 0:392], xsrc0[:, :, 0:392])
            nc.sync.dma_start(xT[:, :, 392:P], xsrc0[:, :, 392:P])
            wpf = cp.tile([128, FP_N], F32)
            nc.sync.dma_start(wpf[:], wpf_d[:])
            wm1_sb = wpb[:, BFP_WM1:BFP_WM1 + 192].rearrange(
                "p (c e) -> p c e", c=3)                     # [128, 3, 64]
            wd_sb = wpb[:, BFP_WD:BFP_WD + 288].rearrange(
                "p (c e) -> p c e", c=3)                     # [128, 3, 96]
            wu_sb = wpb[0:DIM + 1, BFP_WU:BFP_WU + 384].rearrange(
                "p (q c) -> p q c", q=3)                     # [97, 3, 128]; row 96 = bu
            nbm1_sb = wpf[0:E, FP_NBM1:FP_NBM1 + 1]
            pbm1_sb = wpf[0:E, FP_PBM1:FP_PBM1 + 1]
            bd_sb = wpf[0:DIM, FP_BD:FP_BD + 1]
            bpe_sb = wpf[0:E, FP_BPE:FP_BPE + 1]
            wm2_sb = cp.tile([E, E], F32)
            nc.sync.dma_start(wm2_sb[:], wm2_d[:])

            # ---- persistent state ----
            xd_pad = pp.tile([DIM, BL, HP, HP], BF)       # padded gelu(x@Wd+bd)
            w_tap = [pp.tile([DIM, DIM, BL], BF, name=f"w{t}")
                     for t in range(NTAP)]     # conv weights [i,o,b] per tap
            hsum = pp.tile([E, BL], F32)                  # per-sample relu sums
            pvec = pp.tile([EA, BL], BF)                  # (emb+prompt)/WH_SCALE, 1
            y_acc = pp.tile([DIM, BL, P], F32)            # conv partial sums
            y_sb = pp.tile([DIM + 1, BL, P], BF)          # gelu(conv); row 96 = 1

            # zero xd_pad borders (interior is overwritten by the gelu writes)
            nc.gpsimd.memset(xd_pad[:, :, 0:1, :], 0.0)
            nc.gpsimd.memset(xd_pad[:, :, HP - 1:HP, :], 0.0)
            nc.gpsimd.memset(xd_pad[:, :, 1:HP - 1, 0:1], 0.0)
            nc.gpsimd.memset(xd_pad[:, :, 1:HP - 1, HP - 1:HP], 0.0)
            # pvec's bias row multiplies Wh's appended bh row
            nc.gpsimd.memset(pvec[E:EA, :], 1.0)
            # y_sb's bias row (multiplies wu's appended bu row) arrives via
            # DMA: a memset would burn 5us on one partition's free dim
            nc.sync.dma_start(y_sb[DIM:DIM + 1, :, :]
                              .rearrange("p b n -> p (b n)"), ones_d[:])

            # ---- phase 1: per-sample meta sums (xd deferred) ----
            psc_ctx = tc.tile_pool(name="psc", bufs=4, space="PSUM")
            psc = psc_ctx.__enter__()
            ps1m_ctx = tc.tile_pool(name="ps1m", bufs=2, space="PSUM")
            ps1m = ps1m_ctx.__enter__()
            xsrc = x_d.rearrange("(c3 p) n -> p c3 n", p=128)

            def meta_part(b):
                if b > 0:
                    nc.sync.dma_start(xT[:, :, b * P:(b + 1) * P],
                                      xsrc[:, :, b * P:(b + 1) * P])
                ph = ps1m.tile([E, 2, 512], F32, name="ph", tag="ph")
                for h2 in range(2):
                    for c in range(C // 128):
                        nc.tensor.matmul(
                            ph[:, h2, 0:392], wm1_sb[:, c, :],
                            xT[:, c, b * P + h2 * 392: b * P + (h2 + 1) * 392],
                            start=(c == 0), stop=(c == 2))
                # relu(h + bm1) summed over positions, on DVE (keeps ACT free
                # for the gelus); the relu'd values themselves are discarded.
                h_scr = hscr_p.tile([E, P], BF, tag="hscr")
                nc.vector.tensor_scalar(
                    h_scr[:].rearrange("p (h n) -> p h n", h=2),
                    ph[:, :, 0:392], nbm1_sb, pbm1_sb,
                    op0=AluOpType.max, op1=AluOpType.add,
                    accum_out=hsum[:, b:b + 1])

            def xd_part(b):
                for h2 in range(2):
                    px = psc.tile([DIM, 392], F32, name="px", tag="py")
                    for c in range(C // 128):
                        nc.tensor.matmul(
                            px[:], wd_sb[:, c, :],
                            xT[:, c, b * P + h2 * 392: b * P + (h2 + 1) * 392],
                            start=(c == 0), stop=(c == 2))
                    nc.scalar.activation(
                        xd_pad[:, b, 1 + h2 * 14:15 + h2 * 14, 1:HP - 1],
                        px[:].rearrange("p (r c) -> p r c", r=14),
                        AF.Gelu_apprx_sigmoid, bias=bd_sb)

            for b in range(BL):
                meta_part(b)
            # ---- phase 2: pvec = (prompt + emb + bm2) / WH_SCALE ----
            ppm = ps1m.tile([E, 2, 512], F32, name="ppm", tag="ph")
            nc.tensor.matmul(ppm[:, 0, 0:BL], wm2_sb[:], hsum[:],
                             start=True, stop=True)
            nc.vector.tensor_scalar_add(pvec[0:E, :], ppm[:, 0, 0:BL], bpe_sb)
            ps1m_ctx.__exit__(None, None, None)
            if stop_after == "1":
                psc_ctx.__exit__(None, None, None)
                nc.compile(); return nc

            # ---- phase 3 + 4 interleaved: hypernet weights stream per tap;
            # deferred xd matmuls fill the PE while taps arrive; the first
            # conv pass runs as soon as its taps' weights are ready ----
            ps3_ctx = tc.tile_pool(name="ps3", bufs=4, space="PSUM")
            ps3 = ps3_ctx.__enter__()

            # Drains (psum->SBUF copies/adds) are emitted one unit BEHIND
            # their producer matmuls: the tile framework syncs with coarse
            # per-engine counters, so a drain emitted right after its
            # producer makes the NEXT producer wait for it. The lag gives
            # every drain a full unit of slack.
            defer = []

            def flush(keep=0):
                while len(defer) > keep:
                    defer.pop(0)()

            def conv_pass_mm(p, b):
                t0, t1 = TAP_PASS[p], TAP_PASS[p + 1]
                for h2 in range(2):
                    py = psc.tile([DIM, 392], F32, name="py", tag="py")
                    for t in range(t0, t1):
                        dy, dx = t // 3, t % 3
                        nc.tensor.matmul(
                            py[:], w_tap[t][:, :, b],
                            xd_pad[:, b, h2 * 14 + dy: h2 * 14 + dy + 14,
                                   dx:dx + 28],
                            start=(t == t0), stop=(t == t1 - 1))
                    ys = y_acc[:, b, h2 * 392:(h2 + 1) * 392]
                    if p == 0:
                        defer.append(lambda ys=ys, py=py: nc.scalar
                                     .activation(ys, py[:], AF.Copy))
                    else:
                        defer.append(lambda ys=ys, py=py: nc.vector
                                     .tensor_add(ys, py[:], ys))
                    flush(keep=1)

            def hyper_tap(t):
                whc = wh_p.tile([EA, TAPW], WH_DT, tag="whc")
                nc.sync.dma_start(whc[:], wh_d[:, t * TAPW:(t + 1) * TAPW])
                for half in range(2):
                    pwg = ps3.tile([DIM, 48 * BL], F32, name="pwg", tag="pwg")
                    for g in range(48):
                        o = half * 48 + g
                        nc.tensor.matmul(pwg[:, g * BL:(g + 1) * BL],
                                         whc[:, o * DIM:(o + 1) * DIM], pvec[:],
                                         start=True, stop=True)
                    nc.vector.tensor_copy(
                        w_tap[t][:, half * 48:half * 48 + 48, :],
                        pwg[:].rearrange("i (g b) -> i g b", g=48))

            # Emission order interleaves tap matmuls, deferred xd matmuls,
            # and conv pass 0 so the in-order PE never sits behind work whose
            # data hasn't arrived. flush() fully drains the defer queue
            # before any consumer of the deferred writes is emitted.
            # Emission order is queue-aware: the PE waits in-order with a
            # shallow lookahead, so a stalled instruction blocks everything
            # behind it. Work is emitted only once its dependencies are ~2
            # taps behind the Wh stream cursor: xd fills the early stream,
            # conv pass 0 units slot between taps 4..8.
            NT1 = TAP_PASS[1]
            for t in range(NT1):                      # taps 0..3 + xd filler
                xd_part(2 * t)
                if 2 * t + 1 < BL:
                    xd_part(2 * t + 1)
                hyper_tap(t)
            flush()                                   # w taps 0..3 ready
            nconv = [3, 2, 2, 1, 0]
            c0 = 0
            for t in range(NT1, NTAP):                # taps 4..8 + conv filler
                for _ in range(nconv[t - NT1]):
                    if c0 < BL:
                        conv_pass_mm(0, c0)
                        c0 += 1
                hyper_tap(t)
            while c0 < BL:
                conv_pass_mm(0, c0)
                c0 += 1
            flush()                                   # all tap weights ready
            ps3_ctx.__exit__(None, None, None)
            if stop_after == "4":
                psc_ctx.__exit__(None, None, None)
                nc.compile(); return nc

            # ---- final conv pass + gelu + phase 5, pipelined per sample ----
            ps5_ctx = tc.tile_pool(name="ps5", bufs=2, space="PSUM")
            ps5 = ps5_ctx.__enter__()
            odst = out_d.rearrange("(q p) n -> p q n", p=128)

            def phase5(b):
                ob = ob_p.tile([128, C // 128, P], BF, tag="ob")
                for q in range(C // 128):
                    po = ps5.tile([128, 2, 512], F32, name="po", tag="po")
                    for h2 in range(2):
                        nc.tensor.matmul(
                            po[:, h2, 0:392], wu_sb[:, q, :],
                            y_sb[:, b, h2 * 392:(h2 + 1) * 392],
                            start=True, stop=True)

                    def drain(b=b, q=q, po=po, ob=ob):
                        obq = ob[:, q, :].rearrange("p (h n) -> p h n", h=2)
                        if (b + q) % 2 == 0:
                            nc.scalar.activation(obq, po[:, :, 0:392], AF.Copy)
                        else:
                            nc.vector.tensor_copy(obq, po[:, :, 0:392])
                        nc.sync.dma_start(odst[:, q, b * P:(b + 1) * P],
                                          ob[:, q, :])
                    defer.append(drain)
                    flush(keep=1)

            LAG = 2   # samples of lag between last conv pass and phase 5
            for b in range(BL):
                conv_pass_mm(len(TAP_PASS) - 2, b)
                defer.append(lambda b=b: nc.scalar.activation(
                    y_sb[0:DIM, b, :], y_acc[:, b, :], AF.Gelu_apprx_sigmoid))
                if b >= LAG:
                    phase5(b - LAG)
            for b in range(BL - LAG, BL):
                phase5(b)
            if dbg:
                nc.sync.dma_start(y_d[:], y_sb[:].rearrange("i b n -> i (b n)"))
            flush()
            ps5_ctx.__exit__(None, None, None)
            psc_ctx.__exit__(None, None, None)

    nc.compile()
    return nc


_NC_CACHE = None


def _get_nc():
    global _NC_CACHE
    if _NC_CACHE is None:
        _NC_CACHE = build_nc()
    return _NC_CACHE


def _prep_inputs(x, Wd, bd, Wm1, bm1, Wm2, bm2, Wh, bh, emb, Wu, bu):
    """Host-side prep: permute/pack weights, cast, shard x."""
    bf = ml_dtypes.bfloat16
    # Wh columns reordered (o, i, t) -> (t, o, i); scaled for fp8
    whp = np.asarray(Wh, np.float32).reshape(E, DIM, DIM, NTAP)
    whp = whp.transpose(0, 3, 1, 2).reshape(E, WH_COLS)
    bhp = np.asarray(bh, np.float32).reshape(DIM, DIM, NTAP)   # (o, i, t)
    bhp = bhp.transpose(2, 0, 1).reshape(1, WH_COLS)           # (t, o, i)
    whp = np.concatenate([whp * WH_SCALE, bhp], 0)
    whp = np.ascontiguousarray(whp).astype(
        ml_dtypes.float8_e4m3 if WH_FP8 else bf)
    # bf16 pack: wm1 [128, 3*64], wd [128, 3*96], wu (rows 0..95) [*, 384]
    wpb = np.zeros((128, BFP_N), np.float32)
    wpb[:, BFP_WM1:BFP_WM1 + 192] = (
        np.asarray(Wm1, np.float32).reshape(3, 128, E).transpose(1, 0, 2)
        .reshape(128, 192))
    wpb[:, BFP_WD:BFP_WD + 288] = (
        np.asarray(Wd, np.float32).reshape(3, 128, DIM).transpose(1, 0, 2)
        .reshape(128, 288))
    wpb[0:DIM, BFP_WU:BFP_WU + 384] = np.asarray(Wu, np.float32)
    wpb[DIM, BFP_WU:BFP_WU + 384] = np.asarray(bu, np.float32)
    # f32 pack: biases, bu [128, 3], bh [96, 864] (i, (t, o)), wm2 in rows
    # 96..127 of the bh column range
    wpf = np.zeros((128, FP_N), np.float32)
    wpf[0:E, FP_NBM1] = -np.asarray(bm1, np.float32)
    wpf[0:E, FP_PBM1] = P * np.asarray(bm1, np.float32)
    wpf[0:DIM, FP_BD] = np.asarray(bd, np.float32)
    wpf[0:E, FP_BPE] = (np.asarray(bm2, np.float32)
                        + np.asarray(emb, np.float32)) / WH_SCALE
    shared = {
        "wpb": wpb.astype(bf),
        "wpf": wpf,
        "wh": whp,
        "ones": np.ones((1, NPOS), dtype=bf),
        "wm2": np.ascontiguousarray(np.asarray(Wm2, np.float32) / (P * WH_SCALE)),
    }
    xs = np.asarray(x, np.float32).reshape(B, P, C)
    in_maps = []
    for k in range(NCORES):
        m = dict(shared)
        xk = xs[k * BL:(k + 1) * BL].reshape(NPOS, C).T     # [C, NPOS]
        m["x"] = np.ascontiguousarray(xk).astype(bf)
        in_maps.append(m)
    return in_maps


def _run(inputs, **spmd_kwargs):
    nc = _get_nc()
    in_maps = _prep_inputs(**inputs)
    res = run_bass_kernel_spmd(nc, in_maps, core_ids=list(range(NCORES)), **spmd_kwargs)
    outs = []
    for r in res.results:
        o = np.asarray(r["out"], dtype=np.float32)          # [C, NPOS]
        outs.append(o.reshape(C, BL, H, W).transpose(1, 2, 3, 0))
    return np.concatenate(outs, 0), res


def kernel(**inputs) -> np.ndarray:
    out, _ = _run(inputs)
    return out
